# revision 18
# baseline (speedup 1.0000x reference)
"""PointPillarScatter (intersweep, 3 bins) Trainium2 Bass kernel.

Problem: for each of 3 bins, scatter 64000 pillar rows [64 feats] into a
[B=4, C=64, NY=496, NX=432] fp32 canvas at (b, :, y, x); empty cells zero.

Strategy (8 NeuronCores, SPMD), v3:
  - Shard the 12 (bin, b) canvases into 48 quarter-canvases of 124 y-rows;
    6 per core, processed as 3 pairs (A, B).  A pair's canvas is a flat
    [128 ch (A:0-64, B:64-128), 53568 cells] fp16 array; one "group" =
    512 consecutive cells (last group 320), 105 groups per pair.
  - Everything on-device is fp16 (tolerance is 2e-2 rel; fp16 round-trip
    is ~2.4e-4).  The output canvas is stored fp16 and upcast on host;
    this halves the dominant out-DMA bytes vs fp32.
  - Per group one fp16 matmul places the pillars:
      acc[128, W] = lhsT[128, 128].T @ onehot[128, W]     (W = 512/320)
    K rows hold this group's pillars, variably packed (max observed 81
    of 128; features sit in their quarter's 64-col block, other block
    zero).  onehot[k, c] = (rel[k] == c) built by one DVE tensor_scalar
    is_equal against an fp16 iota row (0..511 exact in fp16); unused
    rows have rel = -1 -> all-zero mask rows.  GpSimd is NEVER used:
    its tensor ops run ~20x slower and stall concurrent DVE work
    (shared SBUF ports).
  - lhsT loaded pre-expanded upfront ([128, 315, 128] fp16, 10.3 MB,
    3 persistent tiles) so in-loads don't contend with out-DMAs later;
    expanding compact features on-chip costs far more engine time.
  - PSUM -> SBUF copies convert fp32 -> fp16, 4 groups (one full PSUM
    half) per op to amortize the ~215 ns fixed overhead; most on ACT,
    ~1 in 8 on DVE (balancing ACT copies vs DVE masks).
  - Graded chunk plans: tiny head chunks start the first out-DMA
    ~15 us in; small tail chunks shrink the final drain.  Out-DMAs on
    the sync ring write disjoint cell ranges of the flat pair canvas.
"""

import numpy as np

import concourse.bass as bass
import concourse.tile as tile
from concourse import bacc, mybir
from concourse.bass_utils import run_bass_kernel_spmd

# Problem geometry (hardcoded; kernel.py must be self-contained).
B = 4
C = 64
NX = 432
NY = 496
NBINS = 3
NCORES = 8

NQ = NBINS * B * 4  # 48 quarter-canvases
YQ = NY // 4  # 124 y-rows per quarter
QPC = NQ // NCORES  # 6 quarters per core
PAIRS = QPC // 2  # 3 pairs per core
QCELLS = YQ * NX  # 53568 cells per pair-canvas
GW = 512  # cells per group (PSUM bank)
GPP = (QCELLS + GW - 1) // GW  # 105 groups per pair (last group 320 cells)
RUNT = QCELLS - (GPP - 1) * GW  # 320
G = PAIRS * GPP  # 315 groups per core
KMAX = 128  # pillar slots per group (max observed 81)

# per-pair chunk plans (group counts); last chunk of each pair is the runt
PLANS = [
    [2, 2, 4, 8, 16, 16, 16, 16, 16, 8, 1],
    [16, 16, 16, 16, 16, 16, 8, 1],
    [16, 16, 16, 16, 16, 16, 4, 4, 1],
]
CHMAX = 16
DVE_COPY_EVERY = 7  # every 7th 4-group copy goes to DVE (rest ACT)
# lhst tile split within each pair (group offset, count); aligned with all
# chunk-plan boundaries so every chunk reads one tile
LTSPLIT = [(0, 16), (16, 32), (48, 32), (80, 25)]

_cache = {}


def _build():
    nc = bacc.Bacc(trn_type="TRN2")
    f16 = mybir.dt.float16
    f32 = mybir.dt.float32
    lhst_d = nc.dram_tensor("lhst", [KMAX, G, KMAX], f16, kind="ExternalInput")
    iota_d = nc.dram_tensor("iotat", [KMAX, GW], f16, kind="ExternalInput")
    relc_d = nc.dram_tensor("relc", [KMAX, G], f32, kind="ExternalInput")
    out_d = nc.dram_tensor("out", [PAIRS, KMAX, QCELLS], f16,
                           kind="ExternalOutput")

    with tile.TileContext(nc) as tc:
        with (
            tc.tile_pool(name="const", bufs=1) as constp,
            tc.tile_pool(name="maskp", bufs=8) as maskp,
            tc.tile_pool(name="stage", bufs=3) as stagep,
            tc.tile_pool(name="psum", bufs=2,
                         space=bass.MemorySpace.PSUM) as psump,
        ):
            iota = constp.tile([KMAX, GW], f16, name="iota")
            relc = constp.tile([KMAX, G], f32, name="relc")
            nc.scalar.dma_start(out=iota[:], in_=iota_d[:])
            nc.scalar.dma_start(out=relc[:], in_=relc_d[:])
            # lhst in small tiles; pair 0's load upfront, later pairs' loads
            # spread between early chunks (a burst of dma_starts fills the
            # HWDGE ring and blocks the scalar SEQ - and with it the first
            # PSUM->SBUF copies - for ~10 us)
            lts = {}

            def load_lt(pair, off, cnt):
                lt = constp.tile([KMAX, cnt, KMAX], f16,
                                 name=f"lt{pair}_{off}")
                g0 = pair * GPP + off
                nc.scalar.dma_start(out=lt[:], in_=lhst_d[:, g0:g0 + cnt, :])
                lts[(pair, off)] = lt

            for off, cnt in LTSPLIT:
                load_lt(0, off, cnt)
            deferred = [(p, off, cnt) for p in (1, 2) for off, cnt in LTSPLIT]
            copy_ctr = 0
            chunk_ctr = 0
            for pair in range(PAIRS):
                gp = 0  # group index within pair
                for clen in PLANS[pair]:
                    if chunk_ctr >= 1 and deferred:
                        load_lt(*deferred.pop(0))
                    chunk_ctr += 1
                    lt_off = max(o for o, c in LTSPLIT if o <= gp)
                    lt = lts[(pair, lt_off)]
                    c0 = gp * GW  # cell offset of chunk start
                    ncells = min(QCELLS, (gp + clen) * GW) - c0
                    st = stagep.tile([KMAX, CHMAX * GW], f16, name="st",
                                     tag="st")
                    for qi in range((clen + 3) // 4):
                        w = min(4, clen - 4 * qi)
                        acc = psump.tile([KMAX, 4, GW], f32, name="acc")
                        ccells = 0
                        for l in range(w):
                            g = gp + 4 * qi + l
                            W = RUNT if g == GPP - 1 else GW
                            mask = maskp.tile([KMAX, GW], f16, name="mask")
                            nc.vector.tensor_scalar(
                                out=mask[:, 0:W],
                                in0=iota[:, 0:W],
                                scalar1=relc[:, pair * GPP + g:
                                             pair * GPP + g + 1],
                                scalar2=None,
                                op0=mybir.AluOpType.is_equal,
                            )
                            nc.tensor.matmul(acc[:, l, 0:W],
                                             lt[:, g - lt_off, :],
                                             mask[:, 0:W],
                                             start=True, stop=True)
                            ccells += W
                        off = 4 * qi * GW
                        if ccells == 4 * GW:
                            src = acc[:, :, :]
                        else:
                            src = acc[:, 0:w, 0:GW] if ccells == w * GW else None
                        if src is not None and ccells % GW == 0:
                            dst = st[:, off:off + ccells]
                            copy_ctr += 1
                            if copy_ctr % DVE_COPY_EVERY == 0:
                                nc.vector.tensor_copy(out=dst, in_=src)
                            else:
                                nc.scalar.copy(dst, src)
                        else:
                            # quad containing the runt: copy full groups,
                            # then the runt separately
                            for l in range(w):
                                g = gp + 4 * qi + l
                                W = RUNT if g == GPP - 1 else GW
                                nc.scalar.copy(
                                    st[:, off + l * GW:off + l * GW + W],
                                    acc[:, l, 0:W])
                    out_ap = out_d[pair][:, c0:c0 + ncells]
                    nc.sync.dma_start(out=out_ap, in_=st[:, 0:ncells])
                    gp += clen
    nc.compile()
    return nc


def _pack(inputs):
    """Build per-core lhst/relc arrays (vectorized)."""
    lhst = np.zeros((NCORES, KMAX, G, KMAX), np.float16)
    relc = np.full((NCORES, KMAX, G), -1.0, np.float32)
    iota = np.broadcast_to(np.arange(GW, dtype=np.float16), (KMAX, GW))

    cores = []
    Gs = []
    halves = []
    rels = []
    feats_list = []
    for bin_i in range(NBINS):
        feats = np.asarray(inputs[f"pillar_features_bin_{bin_i}"]).astype(np.float16)
        coords = np.asarray(inputs[f"voxel_coords_bin_{bin_i}"])
        cb = coords[:, 0].astype(np.int64)
        cy = coords[:, 2].astype(np.int64)
        cx = coords[:, 3].astype(np.int64)
        yq = cy // YQ
        q = bin_i * 16 + cb * 4 + yq  # global quarter id
        core, jj = np.divmod(q, QPC)
        pair, half = np.divmod(jj, 2)
        cell = (cy - yq * YQ) * NX + cx
        Garr = pair * GPP + (cell // GW)
        cores.append(core)
        Gs.append(Garr)
        halves.append(half)
        rels.append(cell % GW)
        feats_list.append(feats)

    core = np.concatenate(cores)
    Garr = np.concatenate(Gs)
    half = np.concatenate(halves)
    rel = np.concatenate(rels)
    feats = np.concatenate(feats_list, axis=0)

    # stable order by (core, G, half); slot k = rank within (core, G)
    order = np.lexsort((half, Garr, core))
    core, Garr, half, rel = core[order], Garr[order], half[order], rel[order]
    feats = feats[order]
    key = (core * G + Garr)
    first = np.r_[True, key[1:] != key[:-1]]
    start = np.maximum.accumulate(np.where(first, np.arange(len(key)), 0))
    k = np.arange(len(key)) - start
    if k.max() >= KMAX:
        raise OverflowError(int(k.max()))

    ha = half == 0
    lhst[core[ha], k[ha], Garr[ha], 0:C] = feats[ha]
    hb = ~ha
    lhst[core[hb], k[hb], Garr[hb], C:2 * C] = feats[hb]
    relc[core, k, Garr] = rel

    return [{"lhst": lhst[c], "iotat": iota, "relc": relc[c]}
            for c in range(NCORES)]


def _run(inputs, trace=False):
    if "nc" not in _cache:
        _cache["nc"] = _build()
    nc = _cache["nc"]
    in_maps = _pack(inputs)
    res = run_bass_kernel_spmd(nc, in_maps, core_ids=list(range(NCORES)),
                               trace=trace)
    outs = [np.zeros((B, C, NY, NX), np.float32) for _ in range(NBINS)]
    for core in range(NCORES):
        blk = res.results[core]["out"]  # [PAIRS, 128, QCELLS] f16
        for jj in range(QPC):
            pair, half = divmod(jj, 2)
            q = core * QPC + jj
            bin_i, rem = divmod(q, 16)
            b, yq = divmod(rem, 4)
            outs[bin_i][b, :, YQ * yq:YQ * (yq + 1), :] = (
                blk[pair, half * C:(half + 1) * C].reshape(C, YQ, NX))
    return tuple(outs), res


def kernel(**inputs):
    out, _ = _run(inputs)
    return out


def kernel_traced(**inputs):
    """Like kernel() but also returns BassKernelResults (for test.py)."""
    return _run(inputs, trace=True)


# revision 19
# speedup vs baseline: 1.0892x; 1.0892x over previous
"""PointPillarScatter (intersweep, 3 bins) Trainium2 Bass kernel.

Problem: for each of 3 bins, scatter 64000 pillar rows [64 feats] into a
[B=4, C=64, NY=496, NX=432] fp32 canvas at (b, :, y, x); empty cells zero.

Strategy (8 NeuronCores, SPMD), v3:
  - Shard the 12 (bin, b) canvases into 48 quarter-canvases of 124 y-rows;
    6 per core, processed as 3 pairs (A, B).  A pair's canvas is a flat
    [128 ch (A:0-64, B:64-128), 53568 cells] fp16 array; one "group" =
    512 consecutive cells (last group 320), 105 groups per pair.
  - Everything on-device is fp16 (tolerance is 2e-2 rel; fp16 round-trip
    is ~2.4e-4).  The output canvas is stored fp16 and upcast on host;
    this halves the dominant out-DMA bytes vs fp32.
  - Per group one fp16 matmul places the pillars:
      acc[128, W] = lhsT[128, 128].T @ onehot[128, W]     (W = 512/320)
    K rows hold this group's pillars, variably packed (max observed 81
    of 128; features sit in their quarter's 64-col block, other block
    zero).  onehot[k, c] = (rel[k] == c) built by one DVE tensor_scalar
    is_equal against an fp16 iota row (0..511 exact in fp16); unused
    rows have rel = -1 -> all-zero mask rows.  GpSimd is NEVER used:
    its tensor ops run ~20x slower and stall concurrent DVE work
    (shared SBUF ports).
  - lhsT loaded pre-expanded upfront ([128, 315, 128] fp16, 10.3 MB,
    3 persistent tiles) so in-loads don't contend with out-DMAs later;
    expanding compact features on-chip costs far more engine time.
  - PSUM -> SBUF copies convert fp32 -> fp16, 4 groups (one full PSUM
    half) per op to amortize the ~215 ns fixed overhead; most on ACT,
    ~1 in 8 on DVE (balancing ACT copies vs DVE masks).
  - Graded chunk plans: tiny head chunks start the first out-DMA
    ~15 us in; small tail chunks shrink the final drain.  Out-DMAs on
    the sync ring write disjoint cell ranges of the flat pair canvas.
"""

import numpy as np

import concourse.bass as bass
import concourse.tile as tile
from concourse import bacc, mybir
from concourse.bass_utils import run_bass_kernel_spmd

# Problem geometry (hardcoded; kernel.py must be self-contained).
B = 4
C = 64
NX = 432
NY = 496
NBINS = 3
NCORES = 8

NQ = NBINS * B * 4  # 48 quarter-canvases
YQ = NY // 4  # 124 y-rows per quarter
QPC = NQ // NCORES  # 6 quarters per core
PAIRS = QPC // 2  # 3 pairs per core
QCELLS = YQ * NX  # 53568 cells per pair-canvas
GW = 512  # cells per group (PSUM bank)
GPP = (QCELLS + GW - 1) // GW  # 105 groups per pair (last group 320 cells)
RUNT = QCELLS - (GPP - 1) * GW  # 320
G = PAIRS * GPP  # 315 groups per core
KMAX = 128  # pillar slots per group (max observed 81)

# per-pair chunk plans (group counts); last chunk of each pair is the runt
PLANS = [
    [2, 2, 4, 8, 16, 16, 16, 16, 16, 8, 1],
    [16, 16, 16, 16, 16, 16, 8, 1],
    [16, 16, 16, 16, 16, 16, 4, 4, 1],
]
CHMAX = 16
DVE_COPY_EVERY = 6  # every 6th 4-group copy goes to DVE (rest ACT)
# lhst tile split within each pair (group offset, count); aligned with all
# chunk-plan boundaries so every chunk reads one tile
LTSPLIT = [(0, 16), (16, 32), (48, 32), (80, 25)]

_cache = {}


def _build():
    nc = bacc.Bacc(trn_type="TRN2")
    f16 = mybir.dt.float16
    f32 = mybir.dt.float32
    lhst_d = nc.dram_tensor("lhst", [KMAX, G, KMAX], f16, kind="ExternalInput")
    iota_d = nc.dram_tensor("iotat", [KMAX, GW], f16, kind="ExternalInput")
    relc_d = nc.dram_tensor("relc", [KMAX, G], f32, kind="ExternalInput")
    out_d = nc.dram_tensor("out", [PAIRS, KMAX, QCELLS], f16,
                           kind="ExternalOutput")

    with tile.TileContext(nc) as tc:
        with (
            tc.tile_pool(name="const", bufs=1) as constp,
            tc.tile_pool(name="maskp", bufs=8) as maskp,
            tc.tile_pool(name="stage", bufs=3) as stagep,
            tc.tile_pool(name="psum", bufs=2,
                         space=bass.MemorySpace.PSUM) as psump,
        ):
            iota = constp.tile([KMAX, GW], f16, name="iota")
            relc = constp.tile([KMAX, G], f32, name="relc")
            nc.scalar.dma_start(out=iota[:], in_=iota_d[:])
            nc.scalar.dma_start(out=relc[:], in_=relc_d[:])
            # lhst in small tiles; pair 0's load upfront, later pairs' loads
            # spread between early chunks (a burst of dma_starts fills the
            # HWDGE ring and blocks the scalar SEQ - and with it the first
            # PSUM->SBUF copies - for ~10 us)
            lts = {}

            def load_lt(pair, off, cnt):
                lt = constp.tile([KMAX, cnt, KMAX], f16,
                                 name=f"lt{pair}_{off}")
                g0 = pair * GPP + off
                nc.scalar.dma_start(out=lt[:], in_=lhst_d[:, g0:g0 + cnt, :])
                lts[(pair, off)] = lt

            for off, cnt in LTSPLIT:
                load_lt(0, off, cnt)
            deferred = [(p, off, cnt) for p in (1, 2) for off, cnt in LTSPLIT]
            copy_ctr = 0
            chunk_ctr = 0
            for pair in range(PAIRS):
                gp = 0  # group index within pair
                for clen in PLANS[pair]:
                    if chunk_ctr >= 1 and deferred:
                        load_lt(*deferred.pop(0))
                    chunk_ctr += 1
                    lt_off = max(o for o, c in LTSPLIT if o <= gp)
                    lt = lts[(pair, lt_off)]
                    c0 = gp * GW  # cell offset of chunk start
                    ncells = min(QCELLS, (gp + clen) * GW) - c0
                    st = stagep.tile([KMAX, CHMAX * GW], f16, name="st",
                                     tag="st")
                    for qi in range((clen + 3) // 4):
                        w = min(4, clen - 4 * qi)
                        acc = psump.tile([KMAX, 4, GW], f32, name="acc")
                        ccells = 0
                        for l in range(w):
                            g = gp + 4 * qi + l
                            W = RUNT if g == GPP - 1 else GW
                            mask = maskp.tile([KMAX, GW], f16, name="mask")
                            nc.vector.tensor_scalar(
                                out=mask[:, 0:W],
                                in0=iota[:, 0:W],
                                scalar1=relc[:, pair * GPP + g:
                                             pair * GPP + g + 1],
                                scalar2=None,
                                op0=mybir.AluOpType.is_equal,
                            )
                            nc.tensor.matmul(acc[:, l, 0:W],
                                             lt[:, g - lt_off, :],
                                             mask[:, 0:W],
                                             start=True, stop=True)
                            ccells += W
                        off = 4 * qi * GW
                        if ccells == 4 * GW:
                            src = acc[:, :, :]
                        else:
                            src = acc[:, 0:w, 0:GW] if ccells == w * GW else None
                        if src is not None and ccells % GW == 0:
                            dst = st[:, off:off + ccells]
                            copy_ctr += 1
                            if copy_ctr % DVE_COPY_EVERY == 0:
                                nc.vector.tensor_copy(out=dst, in_=src)
                            else:
                                nc.scalar.copy(dst, src)
                        else:
                            # quad containing the runt: copy full groups,
                            # then the runt separately
                            for l in range(w):
                                g = gp + 4 * qi + l
                                W = RUNT if g == GPP - 1 else GW
                                nc.scalar.copy(
                                    st[:, off + l * GW:off + l * GW + W],
                                    acc[:, l, 0:W])
                    out_ap = out_d[pair][:, c0:c0 + ncells]
                    nc.sync.dma_start(out=out_ap, in_=st[:, 0:ncells])
                    gp += clen
    nc.compile()
    return nc


def _pack(inputs):
    """Build per-core lhst/relc arrays (vectorized)."""
    lhst = np.zeros((NCORES, KMAX, G, KMAX), np.float16)
    relc = np.full((NCORES, KMAX, G), -1.0, np.float32)
    iota = np.broadcast_to(np.arange(GW, dtype=np.float16), (KMAX, GW))

    cores = []
    Gs = []
    halves = []
    rels = []
    feats_list = []
    for bin_i in range(NBINS):
        feats = np.asarray(inputs[f"pillar_features_bin_{bin_i}"]).astype(np.float16)
        coords = np.asarray(inputs[f"voxel_coords_bin_{bin_i}"])
        cb = coords[:, 0].astype(np.int64)
        cy = coords[:, 2].astype(np.int64)
        cx = coords[:, 3].astype(np.int64)
        yq = cy // YQ
        q = bin_i * 16 + cb * 4 + yq  # global quarter id
        core, jj = np.divmod(q, QPC)
        pair, half = np.divmod(jj, 2)
        cell = (cy - yq * YQ) * NX + cx
        Garr = pair * GPP + (cell // GW)
        cores.append(core)
        Gs.append(Garr)
        halves.append(half)
        rels.append(cell % GW)
        feats_list.append(feats)

    core = np.concatenate(cores)
    Garr = np.concatenate(Gs)
    half = np.concatenate(halves)
    rel = np.concatenate(rels)
    feats = np.concatenate(feats_list, axis=0)

    # stable order by (core, G, half); slot k = rank within (core, G)
    order = np.lexsort((half, Garr, core))
    core, Garr, half, rel = core[order], Garr[order], half[order], rel[order]
    feats = feats[order]
    key = (core * G + Garr)
    first = np.r_[True, key[1:] != key[:-1]]
    start = np.maximum.accumulate(np.where(first, np.arange(len(key)), 0))
    k = np.arange(len(key)) - start
    if k.max() >= KMAX:
        raise OverflowError(int(k.max()))

    ha = half == 0
    lhst[core[ha], k[ha], Garr[ha], 0:C] = feats[ha]
    hb = ~ha
    lhst[core[hb], k[hb], Garr[hb], C:2 * C] = feats[hb]
    relc[core, k, Garr] = rel

    return [{"lhst": lhst[c], "iotat": iota, "relc": relc[c]}
            for c in range(NCORES)]


def _run(inputs, trace=False):
    if "nc" not in _cache:
        _cache["nc"] = _build()
    nc = _cache["nc"]
    in_maps = _pack(inputs)
    res = run_bass_kernel_spmd(nc, in_maps, core_ids=list(range(NCORES)),
                               trace=trace)
    outs = [np.zeros((B, C, NY, NX), np.float32) for _ in range(NBINS)]
    for core in range(NCORES):
        blk = res.results[core]["out"]  # [PAIRS, 128, QCELLS] f16
        for jj in range(QPC):
            pair, half = divmod(jj, 2)
            q = core * QPC + jj
            bin_i, rem = divmod(q, 16)
            b, yq = divmod(rem, 4)
            outs[bin_i][b, :, YQ * yq:YQ * (yq + 1), :] = (
                blk[pair, half * C:(half + 1) * C].reshape(C, YQ, NX))
    return tuple(outs), res


def kernel(**inputs):
    out, _ = _run(inputs)
    return out


def kernel_traced(**inputs):
    """Like kernel() but also returns BassKernelResults (for test.py)."""
    return _run(inputs, trace=True)
